# revision 6
# baseline (speedup 1.0000x reference)
"""Distributed Trainium2 kernel for AutoRegressiveGlobalSelfAttention.

B=2, S=2048, D=1024, H=16 (head_dim 64), causal, no 1/sqrt(hd) scale.
Returns (output [B,S,D], proba [B,H,S,S]) like the reference nn.Module.

Sharding: tensor-parallel over heads. Core c owns heads {2c, 2c+1} for both
batch elements. Per core:
  - QKV projections for its 128 output channels (f32r matmuls on the PE)
  - attention in transposed layout S^T[k,q]: exp on ScalarE, causal diagonal
    via affine_select, softmax denominator via a ones-column appended to V
    (the context matmul computes row sums for free)
  - context comes out channel-major [ch, tok] == the AllToAll input layout
  - AllToAll converts head-sharding -> token-sharding (512 tokens/core)
  - output projection with full Wo^T on the 512-token slice
proba is written to DRAM transposed ([k, q]); the causal upper triangle is
never written (output buffers are pre-zeroed by the runtime); the host
restores [q, k] order when assembling the full array.
"""

import numpy as np

import concourse.bacc as bacc
import concourse.tile as tile
import concourse.mybir as mybir
import concourse.bass_utils as bass_utils
from concourse import masks

B, S, D, H = 2, 2048, 1024, 16
HD = D // H          # 64
N_CORES = 8
HPC = H // N_CORES   # heads per core = 2
CPC = HPC * HD       # channels per core = 128
T = B * S            # 4096 flattened tokens
TPC = T // N_CORES   # tokens per core after a2a = 512
STRIP = 512          # q-strip width
N_STRIP = S // STRIP  # 4 strips per batch element
NJT = S // 128       # 16 k-tiles of 128 per batch element

F32 = mybir.dt.float32
F32R = mybir.dt.float32r

_COMPILED = None


def _build():
    nc = bacc.Bacc(
        "TRN2",
        target_bir_lowering=False,
        debug=False,
        enable_asserts=True,
        num_devices=N_CORES,
    )
    # ---- I/O ----
    hst = nc.dram_tensor("hst", [D, T], F32, kind="ExternalInput").ap()       # hs^T
    wqt = nc.dram_tensor("wqt", [D, CPC], F32, kind="ExternalInput").ap()     # Wq_c^T
    wkt = nc.dram_tensor("wkt", [D, CPC], F32, kind="ExternalInput").ap()
    wvt = nc.dram_tensor("wvt", [D, CPC], F32, kind="ExternalInput").ap()
    wot = nc.dram_tensor("wot", [D, D], F32, kind="ExternalInput").ap()       # Wo^T
    bo = nc.dram_tensor("bo", [1, D], F32, kind="ExternalInput").ap()
    out = nc.dram_tensor("out", [TPC, D], F32, kind="ExternalOutput").ap()
    # proba[b*HPC + h][k][q]  (transposed per head-batch)
    proba = nc.dram_tensor("proba", [B * HPC, S, S], F32, kind="ExternalOutput").ap()

    hst_r = hst.bitcast(F32R)
    wqt_r = wqt.bitcast(F32R)
    wkt_r = wkt.bitcast(F32R)
    wvt_r = wvt.bitcast(F32R)
    wot_r = wot.bitcast(F32R)

    with tile.TileContext(nc) as tc:
        with tc.tile_pool(name="const", bufs=1) as constp, \
             tc.tile_pool(name="dram", bufs=1, space="DRAM") as dram:
            ident = constp.tile([128, 128], F32)
            masks.make_identity(nc, ident[:])

            a2a_in = dram.tile([N_CORES, CPC, TPC], F32)
            a2a_out = dram.tile([N_CORES, CPC, TPC], F32)

            for b in range(B):
                _one_batch(nc, tc, b, hst_r, wqt_r, wkt_r, wvt_r, ident,
                           proba, a2a_in)

            # ---- all-to-all: head-sharded ctx -> token-sharded full ctx ----
            nc.gpsimd.collective_compute(
                "AllToAll",
                mybir.AluOpType.bypass,
                replica_groups=[list(range(N_CORES))],
                ins=[a2a_in.opt()],
                outs=[a2a_out.opt()],
            )

            # ---- output projection on the local 512-token slice ----
            with tc.tile_pool(name="osb", bufs=1) as osb, \
                 tc.tile_pool(name="ops", bufs=4, space="PSUM") as ops:
                ctxf = osb.tile([128, N_CORES * TPC], F32R, name="ctxf")
                for kc in range(N_CORES):
                    nc.sync.dma_start(
                        ctxf[:, kc * TPC:(kc + 1) * TPC],
                        a2a_out[kc].bitcast(F32R),
                    )
                wo_sb = osb.tile([128, 8 * D], F32R, name="wo_sb")
                for kc in range(8):
                    nc.sync.dma_start(
                        wo_sb[:, kc * D:(kc + 1) * D],
                        wot_r[kc * 128:(kc + 1) * 128, :],
                    )
                bo_sb = osb.tile([1, D], F32)
                nc.sync.dma_start(bo_sb[:], bo[:])
                bo_b = osb.tile([128, D], F32)
                nc.gpsimd.partition_broadcast(bo_b[:], bo_sb[:])

                for t in range(TPC // 128):  # 4 token tiles
                    for n in range(2):       # 2 x 512 output channels
                        ps = ops.tile([128, 512], F32, tag="ops")
                        for kc in range(8):
                            nc.tensor.matmul(
                                ps[:],
                                ctxf[:, kc * TPC + t * 128: kc * TPC + (t + 1) * 128],
                                wo_sb[:, kc * D + n * 512: kc * D + (n + 1) * 512],
                                start=(kc == 0),
                                stop=(kc == 7),
                            )
                        osb_t = osb.tile([128, 512], F32, tag="osb_t", bufs=4)
                        nc.vector.tensor_add(
                            osb_t[:], ps[:], bo_b[:, n * 512:(n + 1) * 512]
                        )
                        nc.sync.dma_start(
                            out[t * 128:(t + 1) * 128, n * 512:(n + 1) * 512],
                            osb_t[:],
                        )

    nc.compile()
    return nc


def _one_batch(nc, tc, b, hst_r, wqt_r, wkt_r, wvt_r, ident, proba, a2a_in):
    """QKV projections + attention for batch element b (both local heads)."""
    with tc.tile_pool(name=f"qkv{b}", bufs=1) as qkv:
        qt = qkv.tile([128, S], F32R, name=f"qt{b}")     # q^T  [2*64ch, 2048]
        kt = qkv.tile([128, S], F32R, name=f"kt{b}")     # k^T
        # v_aug per head: 16 j-blocks of [128 tok, 65] (64 ch + ones col)
        vaug = [qkv.tile([128, NJT * 65], F32R, name=f"vaug{b}{h}")
                for h in range(HPC)]

        # ---- projections ----
        with tc.tile_pool(name=f"hst{b}", bufs=8) as hsp, \
             tc.tile_pool(name=f"prps{b}", bufs=4, space="PSUM") as prps, \
             tc.tile_pool(name=f"vtmp{b}", bufs=2) as vtmp:
            hs = []
            for kc in range(8):
                t_ = hsp.tile([128, S], F32R, tag="hst")
                nc.sync.dma_start(t_[:], hst_r[kc * 128:(kc + 1) * 128,
                                                b * S:(b + 1) * S])
                hs.append(t_)
            w_sb = vtmp.tile([128, 3 * 8 * CPC], F32R, name=f"w_sb{b}", bufs=1)
            for i, w in enumerate((wqt_r, wkt_r, wvt_r)):
                for kc in range(8):
                    nc.sync.dma_start(
                        w_sb[:, (i * 8 + kc) * CPC:(i * 8 + kc + 1) * CPC],
                        w[kc * 128:(kc + 1) * 128, :],
                    )

            vt = vtmp.tile([128, S], F32, name=f"vt{b}", bufs=1)
            for i, dst in enumerate((qt, kt, vt)):
                for n in range(S // 512):
                    ps = prps.tile([128, 512], F32, tag="prps")
                    for kc in range(8):
                        nc.tensor.matmul(
                            ps[:],
                            w_sb[:, (i * 8 + kc) * CPC:(i * 8 + kc + 1) * CPC],
                            hs[kc][:, n * 512:(n + 1) * 512],
                            start=(kc == 0),
                            stop=(kc == 7),
                        )
                    if i < 2:
                        nc.scalar.copy(dst[:, n * 512:(n + 1) * 512], ps[:])
                    else:
                        nc.vector.tensor_copy(dst[:, n * 512:(n + 1) * 512], ps[:])

            # transpose v^T -> token-major v, pack into v_aug with ones col
            for h in range(HPC):
                nc.vector.memset(
                    vaug[h].bitcast(F32).rearrange("p (j c) -> p j c", c=65)[:, :, 64],
                    1.0,
                )
            for jt in range(NJT):
                pst = prps.tile([128, 128], F32, tag="pst", bufs=2)
                nc.tensor.transpose(pst[:], vt[:, jt * 128:(jt + 1) * 128], ident[:])
                for h in range(HPC):
                    nc.vector.tensor_copy(
                        vaug[h][:, jt * 65: jt * 65 + 64],
                        pst[:, h * 64:(h + 1) * 64],
                    )

        # ---- attention (both heads), q-strips of 512 ----
        with tc.tile_pool(name=f"pt{b}", bufs=36) as ptp, \
             tc.tile_pool(name=f"sm{b}", bufs=8) as smp, \
             tc.tile_pool(name=f"sps{b}", bufs=2, space="PSUM") as sps, \
             tc.tile_pool(name=f"cps{b}", bufs=2, space="PSUM") as cps:
            for i4 in range(N_STRIP):
                q0 = i4 * STRIP
                njs = (q0 + STRIP) // 128  # j-tiles needed for this strip
                ctx_ps = [cps.tile([65, STRIP], F32, tag=f"cps{h}", name=f"ctxps{h}") for h in range(HPC)]
                pts = [[None] * njs for _ in range(HPC)]
                for j in range(njs):
                    for h in range(HPC):
                        sc = sps.tile([128, STRIP], F32, tag=f"sps{h}")
                        nc.tensor.matmul(
                            sc[:],
                            kt[h * 64:(h + 1) * 64, j * 128:(j + 1) * 128],
                            qt[h * 64:(h + 1) * 64, q0:q0 + STRIP],
                            start=True, stop=True,
                        )
                        pt = ptp.tile([128, STRIP], F32R, tag="pt")
                        nc.scalar.activation(
                            pt[:], sc[:], mybir.ActivationFunctionType.Exp
                        )
                        if j * 128 + 127 >= q0:  # diagonal block: causal mask
                            nc.gpsimd.affine_select(
                                out=pt[:], in_=pt[:],
                                compare_op=mybir.AluOpType.is_ge,
                                fill=0.0,
                                base=q0 - j * 128,
                                pattern=[[1, STRIP]],
                                channel_multiplier=-1,
                            )
                        nc.tensor.matmul(
                            ctx_ps[h][:],
                            vaug[h][:, j * 65:(j + 1) * 65],
                            pt[:],
                            start=(j == 0), stop=(j == njs - 1),
                        )
                        pts[h][j] = pt

                for h in range(HPC):
                    recip = smp.tile([1, STRIP], F32, tag="recip")
                    nc.vector.reciprocal(recip[:], ctx_ps[h][64:65, :])
                    recip_b = smp.tile([128, STRIP], F32, tag="recip_b")
                    nc.gpsimd.partition_broadcast(recip_b[:], recip[:])
                    # normalize + store P^T
                    bh = b * HPC + h
                    for j in range(njs):
                        pt = pts[h][j]
                        nc.vector.tensor_mul(pt[:], pt[:], recip_b[:])
                        nc.sync.dma_start(
                            proba[bh, j * 128:(j + 1) * 128, q0:q0 + STRIP],
                            pt.bitcast(F32)[:],
                        )
                    # normalize ctx, ship straight into the a2a input slot
                    ctx_sb = smp.tile([64, STRIP], F32, tag="ctx_sb")
                    nc.vector.tensor_mul(
                        ctx_sb[:], ctx_ps[h][0:64, :], recip_b[0:64, :]
                    )
                    d = b * N_STRIP + i4  # destination core / token slice
                    nc.sync.dma_start(
                        a2a_in[d, h * 64:(h + 1) * 64, :], ctx_sb[:]
                    )


def _get_nc():
    global _COMPILED
    if _COMPILED is None:
        _COMPILED = _build()
    return _COMPILED


def _prep_in_maps(inputs):
    hidden_states = np.asarray(inputs["hidden_states"], dtype=np.float32)
    Wq = np.asarray(inputs["Wq"], dtype=np.float32)
    Wk = np.asarray(inputs["Wk"], dtype=np.float32)
    Wv = np.asarray(inputs["Wv"], dtype=np.float32)
    Wo = np.asarray(inputs["Wo"], dtype=np.float32)
    bo = np.asarray(inputs["bo"], dtype=np.float32)

    hst = np.ascontiguousarray(hidden_states.reshape(T, D).T)   # [D, T]
    wot = np.ascontiguousarray(Wo.T)                            # [D, D]
    bo2 = bo.reshape(1, D)
    in_maps = []
    for c in range(N_CORES):
        sl = slice(c * CPC, (c + 1) * CPC)
        in_maps.append({
            "hst": hst,
            "wqt": np.ascontiguousarray(Wq[sl, :].T),
            "wkt": np.ascontiguousarray(Wk[sl, :].T),
            "wvt": np.ascontiguousarray(Wv[sl, :].T),
            "wot": wot,
            "bo": bo2,
        })
    return in_maps


def kernel(hidden_states, attention_mask, Wq, Wk, Wv, Wo, bo):
    in_maps = _prep_in_maps({
        "hidden_states": hidden_states, "Wq": Wq, "Wk": Wk,
        "Wv": Wv, "Wo": Wo, "bo": bo,
    })
    nc = _get_nc()
    res = bass_utils.run_bass_kernel_spmd(
        nc, in_maps, core_ids=list(range(N_CORES))
    )

    output = np.concatenate(
        [res.results[c]["out"] for c in range(N_CORES)], axis=0
    ).reshape(B, S, D)
    proba = np.empty((B, H, S, S), dtype=np.float32)
    for c in range(N_CORES):
        pr = res.results[c]["proba"]  # [B*HPC, S(k), S(q)]
        for b in range(B):
            for h in range(HPC):
                proba[b, HPC * c + h] = pr[b * HPC + h].T
    return output, proba


# revision 13
# speedup vs baseline: 1.1045x; 1.1045x over previous
"""Distributed Trainium2 kernel for AutoRegressiveGlobalSelfAttention.

B=2, S=2048, D=1024, H=16 (head_dim 64), causal, no 1/sqrt(hd) scale.
Returns (output [B,S,D], proba [B,H,S,S]) like the reference nn.Module.

Sharding: tensor-parallel over heads. Core c owns heads {2c, 2c+1} for both
batch elements. Per core:
  - QKV projections for its 128 output channels (f32r matmuls on the PE)
  - attention in transposed layout S^T[k,q]: exp on ScalarE, causal diagonal
    via affine_select, softmax denominator via a ones-column appended to V
    (the context matmul computes row sums for free)
  - context comes out channel-major [ch, tok] == the AllToAll input layout
  - one AllToAll per batch element (converts head-sharding -> token-sharding,
    256 tokens per core per batch); batch 0's collective and output
    projection overlap batch 1's attention
  - output projection with full Wo^T on the local token slices
proba is written to DRAM transposed ([k, q]); the causal upper triangle is
never written (output buffers are pre-zeroed by the runtime); the host
restores [q, k] order when assembling the full array.
"""

import numpy as np

import concourse.bacc as bacc
import concourse.tile as tile
import concourse.mybir as mybir
import concourse.bass_utils as bass_utils
from concourse import masks

B, S, D, H = 2, 2048, 1024, 16
HD = D // H          # 64
N_CORES = 8
HPC = H // N_CORES   # heads per core = 2
CPC = HPC * HD       # channels per core = 128
T = B * S            # 4096 flattened tokens
SPC = S // N_CORES   # tokens per core per batch after a2a = 256
STRIP = 512          # q-strip width
N_STRIP = S // STRIP  # 4 strips per batch element
NJT = S // 128       # 16 k-tiles of 128 per batch element

F32 = mybir.dt.float32
F32R = mybir.dt.float32r
EXP = mybir.ActivationFunctionType.Exp

_COMPILED = None


def _build():
    nc = bacc.Bacc(
        "TRN2",
        target_bir_lowering=False,
        debug=False,
        enable_asserts=True,
        num_devices=N_CORES,
    )
    # ---- I/O ----
    hst = nc.dram_tensor("hst", [D, T], F32, kind="ExternalInput").ap()       # hs^T
    wqt = nc.dram_tensor("wqt", [D, CPC], F32, kind="ExternalInput").ap()     # Wq_c^T
    wkt = nc.dram_tensor("wkt", [D, CPC], F32, kind="ExternalInput").ap()
    wvt = nc.dram_tensor("wvt", [D, CPC], F32, kind="ExternalInput").ap()
    wot = nc.dram_tensor("wot", [D, D], F32, kind="ExternalInput").ap()       # Wo^T
    bo = nc.dram_tensor("bo", [1, D], F32, kind="ExternalInput").ap()
    # out rows: [batch0 tokens (256), batch1 tokens (256)]
    out = nc.dram_tensor("out", [B * SPC, D], F32, kind="ExternalOutput").ap()
    # proba[b*HPC + h][k][q]  (transposed per head-batch)
    proba = nc.dram_tensor("proba", [B * HPC, S, S], F32, kind="ExternalOutput").ap()

    hst_r = hst.bitcast(F32R)
    wot_r = wot.bitcast(F32R)

    with tile.TileContext(nc) as tc:
        with tc.tile_pool(name="const", bufs=1) as constp, \
             tc.tile_pool(name="dram", bufs=1, space="DRAM") as dram:
            ident = constp.tile([128, 128], F32)
            masks.make_identity(nc, ident[:])
            ones_row = constp.tile([1, 128], F32)
            nc.vector.memset(ones_row[:], 1.0)

            # qkv weights, shared by both batch passes
            w_sb = constp.tile([128, 3 * 8 * CPC], F32R, name="w_sb")
            for i, w in enumerate((wqt, wkt, wvt)):
                for kc in range(8):
                    nc.sync.dma_start(
                        w_sb[:, (i * 8 + kc) * CPC:(i * 8 + kc + 1) * CPC],
                        w.bitcast(F32R)[kc * 128:(kc + 1) * 128, :],
                    )
            # Wo^T + bias, needed only at the tail: off the sync queue
            wo_sb = constp.tile([128, 8 * D], F32R, name="wo_sb")
            for kc in range(8):
                nc.gpsimd.dma_start(
                    wo_sb[:, kc * D:(kc + 1) * D],
                    wot_r[kc * 128:(kc + 1) * 128, :],
                )
            bo_sb = constp.tile([1, D], F32)
            nc.gpsimd.dma_start(bo_sb[:], bo[:])
            bo_b = constp.tile([128, D], F32)
            nc.gpsimd.partition_broadcast(bo_b[:], bo_sb[:])

            a2a = []
            for b in range(B):
                ain = dram.tile([N_CORES, CPC, SPC], F32, name=f"a2a_in{b}")
                aout = dram.tile([N_CORES, CPC, SPC], F32, name=f"a2a_out{b}")
                a2a.append((ain, aout))

            # ---- projections for both batches ----
            qt, kt, vaug = [], [], []
            with tc.tile_pool(name="qkvp", bufs=1) as qkv:
                for b in range(B):
                    qt.append(qkv.tile([128, S], F32R, name=f"qt{b}"))
                    kt.append(qkv.tile([128, S], F32R, name=f"kt{b}"))
                    vaug.append([qkv.tile([128, NJT * 65], F32R, name=f"vaug{b}{h}")
                                 for h in range(HPC)])
                with tc.tile_pool(name="hstp", bufs=8) as hsp, \
                     tc.tile_pool(name="prps", bufs=4, space="PSUM") as prps, \
                     tc.tile_pool(name="vtmp", bufs=2) as vtmp:
                    for b in range(B):
                        _proj_batch(nc, b, hst_r, w_sb, ident, hsp, prps, vtmp,
                                    qt[b], kt[b], vaug[b])

                # ---- attention + per-batch a2a + output projection ----
                with tc.tile_pool(name="ptp", bufs=34) as ptp, \
                     tc.tile_pool(name="smp", bufs=2) as smp, \
                     tc.tile_pool(name="mmps", bufs=2, space="PSUM") as mmps:
                    _attention_batch(nc, 0, qt[0], kt[0], vaug[0], ones_row,
                                     proba, a2a[0][0], ptp, smp, mmps)
                    nc.gpsimd.collective_compute(
                        "AllToAll", mybir.AluOpType.bypass,
                        replica_groups=[list(range(N_CORES))],
                        ins=[a2a[0][0].opt()], outs=[a2a[0][1].opt()],
                    )
                    _attention_batch(nc, 1, qt[1], kt[1], vaug[1], ones_row,
                                     proba, a2a[1][0], ptp, smp, mmps)
                    _outproj_batch(nc, 0, a2a[0][1], wo_sb, bo_b, out,
                                   smp, mmps)
                    nc.gpsimd.collective_compute(
                        "AllToAll", mybir.AluOpType.bypass,
                        replica_groups=[list(range(N_CORES))],
                        ins=[a2a[1][0].opt()], outs=[a2a[1][1].opt()],
                    )
                    _outproj_batch(nc, 1, a2a[1][1], wo_sb, bo_b, out,
                                   smp, mmps)

    nc.compile()
    return nc


def _proj_batch(nc, b, hst_r, w_sb, ident, hsp, prps, vtmp, qt, kt, vaug):
    hs = []
    for kc in range(8):
        t_ = hsp.tile([128, S], F32R, tag="hst", name=f"hs{b}{kc}")
        nc.sync.dma_start(t_[:], hst_r[kc * 128:(kc + 1) * 128,
                                        b * S:(b + 1) * S])
        hs.append(t_)

    vt = vtmp.tile([128, S], F32, name=f"vt{b}", bufs=1)
    for i, dst in enumerate((qt, kt, vt)):
        for n in range(S // 512):
            ps = prps.tile([128, 512], F32, tag="prps", name=f"pp{b}")
            for kc in range(8):
                nc.tensor.matmul(
                    ps[:],
                    w_sb[:, (i * 8 + kc) * CPC:(i * 8 + kc + 1) * CPC],
                    hs[kc][:, n * 512:(n + 1) * 512],
                    start=(kc == 0),
                    stop=(kc == 7),
                )
            if i < 2:
                nc.scalar.copy(dst[:, n * 512:(n + 1) * 512], ps[:])
            else:
                nc.vector.tensor_copy(dst[:, n * 512:(n + 1) * 512], ps[:])

    # transpose v^T -> token-major v, pack into v_aug with ones col
    for h in range(HPC):
        nc.vector.memset(
            vaug[h].bitcast(F32).rearrange("p (j c) -> p j c", c=65)[:, :, 64],
            1.0,
        )
    for jt in range(NJT):
        pst = prps.tile([128, 128], F32, tag="pst", bufs=2, name=f"pst{b}")
        nc.tensor.transpose(pst[:], vt[:, jt * 128:(jt + 1) * 128], ident[:])
        for h in range(HPC):
            nc.vector.tensor_copy(
                vaug[h][:, jt * 65: jt * 65 + 64],
                pst[:, h * 64:(h + 1) * 64],
            )


def _attention_batch(nc, b, qt, kt, vaug, ones_row, proba, a2a_in, ptp, smp, mmps):
    for i4 in range(N_STRIP):
        q0 = i4 * STRIP
        njs = (q0 + STRIP) // 128  # j-tiles needed for this strip
        ctx_ps = [mmps.tile([65, STRIP], F32, tag=f"cps{h}", name=f"cps{b}{h}")
                  for h in range(HPC)]
        pts = [[None] * njs for _ in range(HPC)]
        for j in range(njs):
            scs = []
            for h in range(HPC):  # adjacent emission -> PE row-group packing
                sc = mmps.tile([128, STRIP], F32, tag=f"sps{h}", name=f"sc{b}{h}")
                nc.tensor.matmul(
                    sc[:],
                    kt[h * 64:(h + 1) * 64, j * 128:(j + 1) * 128],
                    qt[h * 64:(h + 1) * 64, q0:q0 + STRIP],
                    start=True, stop=True,
                )
                scs.append(sc)
            for h in range(HPC):
                pt = ptp.tile([128, STRIP], F32R, tag="pt", name=f"pt{b}{h}")
                nc.scalar.activation(pt[:], scs[h][:], EXP)
                if j * 128 + 127 >= q0:  # diagonal block: causal mask
                    nc.gpsimd.affine_select(
                        out=pt[:], in_=pt[:],
                        compare_op=mybir.AluOpType.is_ge,
                        fill=0.0,
                        base=q0 - j * 128,
                        pattern=[[1, STRIP]],
                        channel_multiplier=-1,
                    )
                pts[h][j] = pt
            for h in range(HPC):
                nc.tensor.matmul(
                    ctx_ps[h][:],
                    vaug[h][:, j * 65:(j + 1) * 65],
                    pts[h][j][:],
                    start=(j == 0), stop=(j == njs - 1),
                )

        for h in range(HPC):
            recip = smp.tile([1, STRIP], F32, tag="recip", name=f"rc{b}{h}", bufs=2)
            nc.vector.reciprocal(recip[:], ctx_ps[h][64:65, :])
            rb_sb = smp.tile([128, STRIP], F32, tag="rb_sb", name=f"rs{b}{h}", bufs=3)
            nc.gpsimd.partition_broadcast(rb_sb[:], recip[:])
            bh = b * HPC + h
            for j in range(njs):
                pt = pts[h][j]
                nc.vector.tensor_mul(pt[:], pt[:], rb_sb[:])
                nc.sync.dma_start(
                    proba[bh, j * 128:(j + 1) * 128, q0:q0 + STRIP],
                    pt.bitcast(F32)[:],
                )
            # normalize ctx, ship straight into the a2a input slots
            ctx_sb = smp.tile([64, STRIP], F32, tag="ctx_sb", name=f"cs{b}{h}", bufs=3)
            nc.vector.tensor_mul(ctx_sb[:], ctx_ps[h][0:64, :], rb_sb[0:64, :])
            for half in range(2):  # strip covers two 256-token a2a slices
                d = 2 * i4 + half
                nc.sync.dma_start(
                    a2a_in[d, h * 64:(h + 1) * 64, :],
                    ctx_sb[:, half * SPC:(half + 1) * SPC],
                )


def _outproj_batch(nc, b, a2a_out, wo_sb, bo_b, out, smp, mmps):
    ctxf = smp.tile([128, 8 * SPC], F32R, tag="ctxf", name=f"ctxf{b}", bufs=2)
    for kc in range(N_CORES):
        nc.sync.dma_start(
            ctxf[:, kc * SPC:(kc + 1) * SPC],
            a2a_out[kc].bitcast(F32R),
        )
    for t in range(SPC // 128):  # 2 token tiles
        for n in range(2):       # 2 x 512 output channels
            ps = mmps.tile([128, 512], F32, tag=f"sps{n}", name=f"op{b}{n}")
            for kc in range(8):
                nc.tensor.matmul(
                    ps[:],
                    ctxf[:, kc * SPC + t * 128: kc * SPC + (t + 1) * 128],
                    wo_sb[:, kc * D + n * 512: kc * D + (n + 1) * 512],
                    start=(kc == 0),
                    stop=(kc == 7),
                )
            ot = smp.tile([128, 512], F32, tag="osb_t", name=f"ot{b}{n}", bufs=2)
            nc.vector.tensor_add(ot[:], ps[:], bo_b[:, n * 512:(n + 1) * 512])
            nc.sync.dma_start(
                out[b * SPC + t * 128: b * SPC + (t + 1) * 128,
                    n * 512:(n + 1) * 512],
                ot[:],
            )


def _get_nc():
    global _COMPILED
    if _COMPILED is None:
        _COMPILED = _build()
    return _COMPILED


def _prep_in_maps(inputs):
    hidden_states = np.asarray(inputs["hidden_states"], dtype=np.float32)
    Wq = np.asarray(inputs["Wq"], dtype=np.float32)
    Wk = np.asarray(inputs["Wk"], dtype=np.float32)
    Wv = np.asarray(inputs["Wv"], dtype=np.float32)
    Wo = np.asarray(inputs["Wo"], dtype=np.float32)
    bo = np.asarray(inputs["bo"], dtype=np.float32)

    hst = np.ascontiguousarray(hidden_states.reshape(T, D).T)   # [D, T]
    wot = np.ascontiguousarray(Wo.T)                            # [D, D]
    bo2 = bo.reshape(1, D)
    in_maps = []
    for c in range(N_CORES):
        sl = slice(c * CPC, (c + 1) * CPC)
        in_maps.append({
            "hst": hst,
            "wqt": np.ascontiguousarray(Wq[sl, :].T),
            "wkt": np.ascontiguousarray(Wk[sl, :].T),
            "wvt": np.ascontiguousarray(Wv[sl, :].T),
            "wot": wot,
            "bo": bo2,
        })
    return in_maps


def kernel(hidden_states, attention_mask, Wq, Wk, Wv, Wo, bo):
    in_maps = _prep_in_maps({
        "hidden_states": hidden_states, "Wq": Wq, "Wk": Wk,
        "Wv": Wv, "Wo": Wo, "bo": bo,
    })
    nc = _get_nc()
    res = bass_utils.run_bass_kernel_spmd(
        nc, in_maps, core_ids=list(range(N_CORES))
    )

    # out rows per core: [b0 tokens 256c..256c+256, b1 tokens 256c..]
    output = np.empty((B, S, D), dtype=np.float32)
    for c in range(N_CORES):
        o = res.results[c]["out"]
        for b in range(B):
            output[b, c * SPC:(c + 1) * SPC] = o[b * SPC:(b + 1) * SPC]
    proba = np.empty((B, H, S, S), dtype=np.float32)
    for c in range(N_CORES):
        pr = res.results[c]["proba"]  # [B*HPC, S(k), S(q)]
        for b in range(B):
            for h in range(HPC):
                proba[b, HPC * c + h] = pr[b * HPC + h].T
    return output, proba
